# revision 7
# baseline (speedup 1.0000x reference)
"""Multi-Head Latent Attention (MLA) Trainium2 Bass kernel.

Sharding: 8 cores = (batch b in 0..4) x (head-group g in 0..2).
Each core computes one batch's full low-rank down-projections (duplicated
across the pair), its 8 heads' up-projections + attention, and a partial
W_o row-parallel output.  Host sums the two partials per batch and
transposes compressed_kv back to natural layout.

All activations live TRANSPOSED on-chip ([feature partitions, seq free]),
weights stay stationary, matmuls run in fp32r (tf32-like, full PE rate at
N=512).  RoPE's rotate_half is folded into duplicated/sign-permuted weight
columns so everything stays partition-aligned; softmax runs max-free
(scores are tiny) in the transposed layout with the denominator produced
by an extra ones-column in V.
"""

import math

import numpy as np

import concourse.bass as bass
import concourse.tile as tile
from concourse import bacc, mybir
from concourse.bass import ts
from concourse.bass_utils import run_bass_kernel_spmd

F32 = mybir.dt.float32
F32R = mybir.dt.float32r
AF = mybir.ActivationFunctionType

P = 128
S = 1024
D = 1536
RQ = 768          # q low-rank dim
RKV = 1024        # kv low-rank dim
NOPE = 48
ROPE = 48
RH = 24           # rope half
DH = 96
HL = 8            # heads per core
NB = 2            # 512-wide seq blocks
KT = 8            # 128-wide key tiles
DO = 12           # D // 128
MQ = 6            # RQ // 128
MKV = 8           # RKV // 128
LN_EPS = 1e-5
SCALE = 1.0 / math.sqrt(DH)

_PROG_CACHE = {}


def _build_program():
    nc = bacc.Bacc("TRN2", target_bir_lowering=False, debug=False)

    def din(name, shape):
        return nc.dram_tensor(name, shape, F32, kind="ExternalInput").ap()

    xt = din("xt", [D, S])
    w_dq = din("w_dq", [D, RQ])
    w_uq_a = din("w_uq_a", [RQ, HL * 112])
    w_uq_b = din("w_uq_b", [RQ, 4 * 112])
    w_dkv_e = din("w_dkv_e", [D, 1136])
    w_ukv_n = din("w_ukv_n", [RKV, HL * 64])
    w_ukv_v = din("w_ukv_v", [RKV, HL * 96])
    w_o_t = din("w_o_t", [RQ, D])
    q_ln_w_t = din("q_ln_w_t", [P, MQ])
    q_ln_b_t = din("q_ln_b_t", [P, MQ])
    kv_ln_w_t = din("kv_ln_w_t", [P, MKV])
    kv_ln_b_t = din("kv_ln_b_t", [P, MKV])
    cos_t = din("cos_t", [P, S])
    sin_t = din("sin_t", [P, S])
    mask_t = din("mask_t", [P, 4 * 512])

    out_p = nc.dram_tensor("out_p", [S, D], F32, kind="ExternalOutput").ap()
    ckv_t = nc.dram_tensor("ckv_t", [RKV + ROPE, S], F32, kind="ExternalOutput").ap()

    with tile.TileContext(nc) as tc:
        # ---------------- constants ----------------
        constp = tc.alloc_tile_pool(name="const", bufs=1)
        cos_sb = constp.tile([P, S], F32)
        nc.sync.dma_start(cos_sb[:], cos_t[:])
        sin_sb = constp.tile([P, S], F32)
        nc.sync.dma_start(sin_sb[:], sin_t[:])
        mask_sb = constp.tile([P, 4, 512], F32)
        nc.sync.dma_start(mask_sb[:], mask_t.rearrange("p (d n) -> p d n", d=4))
        lnqw = constp.tile([P, MQ], F32)
        nc.sync.dma_start(lnqw[:], q_ln_w_t[:])
        lnqb = constp.tile([P, MQ], F32)
        nc.sync.dma_start(lnqb[:], q_ln_b_t[:])
        lnkw = constp.tile([P, MKV], F32)
        nc.sync.dma_start(lnkw[:], kv_ln_w_t[:])
        lnkb = constp.tile([P, MKV], F32)
        nc.sync.dma_start(lnkb[:], kv_ln_b_t[:])
        ones_f = constp.tile([P, 1], F32)
        nc.vector.memset(ones_f[:], 1.0)
        ones_r = constp.tile([P, 1], F32R)
        nc.vector.tensor_copy(ones_r[:], ones_f[:])
        eps_t = constp.tile([1, 1], F32)
        nc.vector.memset(eps_t[:], LN_EPS)

        # persistent activations
        ckvp = tc.alloc_tile_pool(name="ckv", bufs=1)
        ckv = ckvp.tile([P, 9, S], F32R)       # rows m<8: kv latent; m=8: rope
        krp = tc.alloc_tile_pool(name="krope", bufs=1)
        kr_a = krp.tile([112, S], F32R)        # [64:112] = raw rope-A of K
        kr_b = krp.tile([112, S], F32R)        # [64:112] = raw rope-B of K

        xtp = tc.alloc_tile_pool(name="xt", bufs=1, side="right")
        xts = xtp.tile([P, DO, S], F32R)
        for o in range(DO):
            nc.gpsimd.dma_start(xts[:, o, :], xt[P * o:P * o + P, :])

        psdp = tc.alloc_tile_pool(name="psdp", bufs=3, space="PSUM")

        # ---------------- phase A: cq = (X @ W_dq)^T ---------------------
        cqp = tc.alloc_tile_pool(name="cq", bufs=1)
        cq = cqp.tile([P, MQ, S], F32R)
        wqp = tc.alloc_tile_pool(name="wdq", bufs=1, side="right")
        wdq = wqp.tile([P, DO, RQ], F32R)
        for o in range(DO):
            nc.gpsimd.dma_start(wdq[:, o, :], w_dq[P * o:P * o + P, :])
        for m in range(MQ):
            for n in range(NB):
                ps = psdp.tile([P, 512], F32, tag="dp")
                for o in range(DO):
                    nc.tensor.matmul(
                        ps[:], wdq[:, o, ts(m, 128)], xts[:, o, ts(n, 512)],
                        start=(o == 0), stop=(o == DO - 1))
                nc.vector.tensor_copy(cq[:, m, ts(n, 512)], ps[:])
        wqp.release()

        # ---------------- phase B: ckv = (X @ W_dkv)^T -------------------
        wp = tc.alloc_tile_pool(name="wdkv", bufs=1, side="right")
        wdkv = wp.tile([P, DO, 1136], F32R)
        for o in range(DO):
            nc.gpsimd.dma_start(wdkv[:, o, :], w_dkv_e[P * o:P * o + P, :])
        for m in range(9):
            rows = 128 if m < 8 else 112
            for n in range(NB):
                ps = psdp.tile([P, 512], F32, tag="dp")
                for o in range(DO):
                    nc.tensor.matmul(
                        ps[0:rows], wdkv[:, o, 128 * m:128 * m + rows],
                        xts[:, o, ts(n, 512)], start=(o == 0), stop=(o == DO - 1))
                if m < 8:
                    nc.vector.tensor_copy(ckv[0:rows, m, ts(n, 512)], ps[0:rows])
                else:
                    nc.vector.tensor_copy(ckv[0:48, 8, ts(n, 512)], ps[0:48])
                    nc.scalar.copy(kr_a[64:112, ts(n, 512)], ps[0:48])
                    nc.scalar.copy(kr_b[64:112, ts(n, 512)], ps[64:112])
        # compressed_kv out (pre-LN, rope raw)
        for m in range(MKV):
            nc.sync.dma_start(ckv_t[P * m:P * m + P, :], ckv[:, m, :].bitcast(F32))
        nc.sync.dma_start(ckv_t[RKV:RKV + ROPE, :], ckv[0:48, 8, :].bitcast(F32))

        wp.release()
        xtp.release()
        psdp.release()

        # ---------------- layernorm (transposed, partition-reduce) ------
        lntmp = tc.alloc_tile_pool(name="lntmp", bufs=2)
        lnbc = tc.alloc_tile_pool(name="lnbc", bufs=1)
        lnvec = tc.alloc_tile_pool(name="lnvec", bufs=1)
        psst = tc.alloc_tile_pool(name="psst", bufs=1, space="PSUM")

        def ln_t(act, M, wln, bln):
            ps_sum = [psst.tile([1, 512], F32, tag=f"st{n}", name=f"st{n}") for n in range(NB)]
            ps_sq = [psst.tile([1, 512], F32, tag=f"sq{n}", name=f"sq{n}") for n in range(NB)]
            for m in range(M):
                for n in range(NB):
                    sq = lntmp.tile([P, 512], F32R, tag="sq")
                    nc.vector.tensor_mul(sq[:], act[:, m, ts(n, 512)],
                                         act[:, m, ts(n, 512)])
                    nc.tensor.matmul(ps_sum[n][:], ones_r[:], act[:, m, ts(n, 512)],
                                     start=(m == 0), stop=(m == M - 1))
                    nc.tensor.matmul(ps_sq[n][:], ones_r[:], sq[:],
                                     start=(m == 0), stop=(m == M - 1))
            mean = lnvec.tile([1, S], F32, tag="mean")
            var = lnvec.tile([1, S], F32, tag="var")
            msq = lnvec.tile([1, S], F32, tag="msq")
            for n in range(NB):
                nc.vector.tensor_scalar_mul(mean[:, ts(n, 512)], ps_sum[n][:], 1.0 / (M * P))
                nc.vector.tensor_scalar_mul(var[:, ts(n, 512)], ps_sq[n][:], 1.0 / (M * P))
            nc.vector.tensor_mul(msq[:], mean[:], mean[:])
            nc.vector.tensor_sub(var[:], var[:], msq[:])
            nc.scalar.activation(var[:], var[:], AF.Sqrt, bias=eps_t[:])
            nc.vector.reciprocal(var[:], var[:])
            mb = lnbc.tile([P, S], F32, tag="mb")
            rb = lnbc.tile([P, S], F32, tag="rb")
            nc.gpsimd.partition_broadcast(mb[:], mean[:])
            nc.gpsimd.partition_broadcast(rb[:], var[:])
            for m in range(M):
                t = lntmp.tile([P, S], F32, tag="lnt")
                nc.vector.tensor_sub(t[:], act[:, m, :], mb[:])
                nc.vector.tensor_mul(t[:], t[:], rb[:])
                nc.scalar.activation(act[:, m, :], t[:], AF.Identity,
                                     scale=wln[:, m:m + 1], bias=bln[:, m:m + 1])

        ln_t(cq, MQ, lnqw, lnqb)          # q latent LN (in place)
        ln_t(ckv, MKV, lnkw, lnkb)        # kv latent LN (in place, after ckv out)

        lnvec.release()
        lnbc.release()
        lntmp.release()
        psst.release()

        # ---------------- phase D: Q up-proj + rope ----------------------
        qkp = tc.alloc_tile_pool(name="qk", bufs=1, side="right")
        q_sb = qkp.tile([112, HL, S], F32R)
        k_sb = qkp.tile([112, HL, S], F32R)

        psq = tc.alloc_tile_pool(name="psq", bufs=3, space="PSUM")
        wup = tc.alloc_tile_pool(name="wuq", bufs=1, side="right")
        wuqa = wup.tile([P, MQ, HL * 112], F32R)
        for o in range(MQ):
            nc.gpsimd.dma_start(wuqa[:, o, :], w_uq_a[P * o:P * o + P, :])
        wuqb = wup.tile([P, MQ, 4 * 112], F32R)
        for o in range(MQ):
            nc.gpsimd.dma_start(wuqb[:, o, :], w_uq_b[P * o:P * o + P, :])

        qtmp = tc.alloc_tile_pool(name="qtmp", bufs=3, side="right")
        for h in range(HL):
            for n in range(NB):
                ps = psq.tile([P, 512], F32, tag="up")
                for o in range(MQ):
                    nc.tensor.matmul(ps[0:112], wuqa[:, o, 112 * h:112 * h + 112],
                                     cq[:, o, ts(n, 512)],
                                     start=(o == 0), stop=(o == MQ - 1))
                nc.vector.tensor_copy(q_sb[0:64, h, ts(n, 512)], ps[0:64])
                nc.vector.tensor_mul(q_sb[64:112, h, ts(n, 512)], ps[64:112],
                                     cos_sb[64:112, ts(n, 512)])
        for p4 in range(4):
            for n in range(NB):
                ps = psq.tile([P, 512], F32, tag="up")
                for o in range(MQ):
                    nc.tensor.matmul(ps[0:112], wuqb[:, o, 112 * p4:112 * p4 + 112],
                                     cq[:, o, ts(n, 512)],
                                     start=(o == 0), stop=(o == MQ - 1))
                for hh, base in ((2 * p4, 0), (2 * p4 + 1, 64)):
                    tb = qtmp.tile([112, 512], F32R, tag="tb")
                    nc.vector.tensor_mul(tb[64:112, :], ps[base:base + 48],
                                         sin_sb[64:112, ts(n, 512)])
                    nc.vector.tensor_add(q_sb[64:112, hh, ts(n, 512)],
                                         q_sb[64:112, hh, ts(n, 512)], tb[64:112, :])
        qtmp.release()
        wup.release()
        cqp.release()

        # ---------------- phase F: K rope ---------------------------------
        nc.vector.tensor_mul(kr_a[64:112, :], kr_a[64:112, :], cos_sb[64:112, :])
        nc.vector.tensor_mul(kr_b[64:112, :], kr_b[64:112, :], sin_sb[64:112, :])
        nc.vector.tensor_add(kr_a[64:112, :], kr_a[64:112, :], kr_b[64:112, :])
        for h in range(HL):
            nc.vector.tensor_copy(k_sb[64:112, h, :], kr_a[64:112, :])
        krp.release()

        # ---------------- phase G: K_nope up-proj -------------------------
        wnp = tc.alloc_tile_pool(name="wun", bufs=1, side="right")
        wun = wnp.tile([P, MKV, HL * 64], F32R)
        for o in range(MKV):
            nc.gpsimd.dma_start(wun[:, o, :], w_ukv_n[P * o:P * o + P, :])
        for mt in range(4):
            for n in range(NB):
                ps = psq.tile([P, 512], F32, tag="up")
                for o in range(MKV):
                    nc.tensor.matmul(ps[:], wun[:, o, ts(mt, 128)],
                                     ckv[:, o, ts(n, 512)],
                                     start=(o == 0), stop=(o == MKV - 1))
                nc.vector.tensor_copy(k_sb[0:64, 2 * mt, ts(n, 512)], ps[0:64])
                nc.vector.tensor_copy(k_sb[0:64, 2 * mt + 1, ts(n, 512)], ps[64:128])
        wnp.release()

        # ---------------- phase H: V up-proj (natural layout) -------------
        vp = tc.alloc_tile_pool(name="v", bufs=1, side="right")
        v_sb = vp.tile([P, KT, HL * 97], F32R)
        wvp = tc.alloc_tile_pool(name="wuv", bufs=1, side="right")
        wuv = wvp.tile([P, MKV, HL * 96], F32R)
        for o in range(MKV):
            nc.gpsimd.dma_start(wuv[:, o, :], w_ukv_v[P * o:P * o + P, :])
        for t in range(KT):
            for h in range(HL):
                nc.vector.tensor_copy(v_sb[:, t, 97 * h + 96:97 * h + 97], ones_f[:])
            for c0, cn in ((0, 512), (512, 256)):
                ps = psq.tile([P, 512], F32, tag="up")
                for o in range(MKV):
                    nc.tensor.matmul(ps[:, 0:cn], ckv[:, o, ts(t, 128)],
                                     wuv[:, o, c0:c0 + cn],
                                     start=(o == 0), stop=(o == MKV - 1))
                for h in range(HL):
                    s0, s1 = max(96 * h, c0), min(96 * h + 96, c0 + cn)
                    if s0 < s1:
                        nc.vector.tensor_copy(
                            v_sb[:, t, 97 * h + s0 - 96 * h:97 * h + s1 - 96 * h],
                            ps[:, s0 - c0:s1 - c0])
        wvp.release()
        ckvp.release()
        psq.release()

        # ---------------- phase I: attention ------------------------------
        pssc = tc.alloc_tile_pool(name="pssc", bufs=3, space="PSUM")
        pspv = tc.alloc_tile_pool(name="pspv", bufs=2, space="PSUM")
        atp = tc.alloc_tile_pool(name="attnT", bufs=1)
        attnT = atp.tile([P, MQ, S], F32R)
        ppool = tc.alloc_tile_pool(name="p", bufs=6)
        denp = tc.alloc_tile_pool(name="den", bufs=3)
        for h in range(HL):
            for j in range(NB):
                pv = pspv.tile([P, 512], F32, tag="pv")
                nkt = 4 * (j + 1)
                for t in range(nkt):
                    sp = pssc.tile([P, 512], F32, tag="s")
                    nc.tensor.matmul(sp[:], k_sb[0:112, h, ts(t, 128)],
                                     q_sb[0:112, h, ts(j, 512)],
                                     start=True, stop=True)
                    pt = ppool.tile([P, 512], F32R, tag="p")
                    nc.scalar.activation(pt[:], sp[:], AF.Exp, scale=SCALE)
                    d = t - 4 * j
                    if d >= 0:
                        nc.vector.tensor_mul(pt[:], pt[:], mask_sb[:, d, :])
                    nc.tensor.matmul(pv[0:97], v_sb[:, t, 97 * h:97 * h + 97], pt[:],
                                     start=(t == 0), stop=(t == nkt - 1))
                den = denp.tile([1, 512], F32, tag="den")
                nc.vector.tensor_copy(den[:], pv[96:97, :])
                nc.vector.reciprocal(den[:], den[:])
                rb = denp.tile([P, 512], F32, tag="rb")
                nc.gpsimd.partition_broadcast(rb[:], den[:])
                f0 = 96 * h
                for z in range(3):          # 32-row pieces keep all partition
                    fs = f0 + 32 * z        # bases 32-aligned and counts <= 32
                    nc.vector.tensor_mul(
                        attnT[fs % 128:fs % 128 + 32, fs // 128, ts(j, 512)],
                        pv[32 * z:32 * z + 32, :], rb[fs % 128:fs % 128 + 32, :])

        denp.release()
        ppool.release()
        pspv.release()
        vp.release()
        qkp.release()

        # ---------------- phase J: out = attnT.T @ W_o^T partial ---------
        wop = tc.alloc_tile_pool(name="wo", bufs=1, side="right")
        wot = wop.tile([P, MQ, D], F32R)
        for o in range(MQ):
            nc.gpsimd.dma_start(wot[:, o, :], w_o_t[P * o:P * o + P, :])
        stout = tc.alloc_tile_pool(name="stout", bufs=3)
        for m in range(8):
            for n in range(3):
                ps = pssc.tile([P, 512], F32, tag="s")
                for o in range(MQ):
                    nc.tensor.matmul(ps[:], attnT[:, o, ts(m, 128)],
                                     wot[:, o, ts(n, 512)],
                                     start=(o == 0), stop=(o == MQ - 1))
                st = stout.tile([P, 512], F32, tag="st")
                nc.vector.tensor_copy(st[:], ps[:])
                nc.sync.dma_start(out_p[ts(m, 128), ts(n, 512)], st[:])
        stout.release()
        wop.release()
        atp.release()
        pssc.release()
        constp.release()

    nc.compile()
    return nc


def _rot_cols(a):
    return np.concatenate([-a[:, RH:2 * RH], a[:, 0:RH]], axis=1)


def _host_prep(inputs):
    feats = np.ascontiguousarray(np.asarray(inputs["features"], np.float32))
    W_dq = np.asarray(inputs["W_dq"], np.float32)
    W_uq = np.asarray(inputs["W_uq"], np.float32)
    q_ln_w = np.asarray(inputs["q_ln_w"], np.float32)
    q_ln_b = np.asarray(inputs["q_ln_b"], np.float32)
    W_dkv = np.asarray(inputs["W_dkv"], np.float32)
    W_ukv = np.asarray(inputs["W_ukv"], np.float32)
    kv_ln_w = np.asarray(inputs["kv_ln_w"], np.float32)
    kv_ln_b = np.asarray(inputs["kv_ln_b"], np.float32)
    W_o = np.asarray(inputs["W_o"], np.float32)
    cos_tab = np.asarray(inputs["cos_cached"], np.float32)[0, 0, :S, 0:RH]
    sin_tab = np.asarray(inputs["sin_cached"], np.float32)[0, 0, :S, 0:RH]

    cos_t = np.zeros((P, S), np.float32)
    sin_t = np.zeros((P, S), np.float32)
    jj = (np.arange(64, 112) - 64) % RH
    cos_t[64:112] = cos_tab[:, jj].T
    sin_t[64:112] = sin_tab[:, jj].T

    f = np.arange(512)
    p = np.arange(P)
    mask_t = np.concatenate(
        [(f[None, :] >= (p[:, None] + 128 * d)).astype(np.float32) for d in range(4)],
        axis=1)

    def part_major(v, m):
        return np.ascontiguousarray(v.reshape(m, P).T)

    wdkv_e = np.zeros((D, 1136), np.float32)
    wdkv_e[:, :RKV] = W_dkv[:, :RKV]
    wdkv_e[:, RKV:RKV + ROPE] = W_dkv[:, RKV:RKV + ROPE]
    wdkv_e[:, 1088:1136] = _rot_cols(W_dkv[:, RKV:RKV + ROPE])

    shared = {
        "w_dq": np.ascontiguousarray(W_dq),
        "w_dkv_e": wdkv_e,
        "q_ln_w_t": part_major(q_ln_w, MQ),
        "q_ln_b_t": part_major(q_ln_b, MQ),
        "kv_ln_w_t": part_major(kv_ln_w, MKV),
        "kv_ln_b_t": part_major(kv_ln_b, MKV),
        "cos_t": cos_t,
        "sin_t": sin_t,
        "mask_t": np.ascontiguousarray(mask_t),
    }

    per_g = []
    for g in range(2):
        h0 = HL * g
        wuqa = np.zeros((RQ, HL * 112), np.float32)
        wuqb = np.zeros((RQ, 4 * 112), np.float32)
        wun = np.zeros((RKV, HL * 64), np.float32)
        wuv = np.zeros((RKV, HL * 96), np.float32)
        for h in range(HL):
            hg = h0 + h
            wuqa[:, 112 * h:112 * h + NOPE] = W_uq[:, DH * hg:DH * hg + NOPE]
            wuqa[:, 112 * h + 64:112 * h + 112] = W_uq[:, DH * hg + NOPE:DH * hg + DH]
            wun[:, 64 * h:64 * h + NOPE] = W_ukv[:, 144 * hg:144 * hg + NOPE]
            wuv[:, 96 * h:96 * h + 96] = W_ukv[:, 144 * hg + NOPE:144 * hg + 144]
        for p4 in range(4):
            for k, h in enumerate((2 * p4, 2 * p4 + 1)):
                hg = h0 + h
                wuqb[:, 112 * p4 + 64 * k:112 * p4 + 64 * k + 48] = _rot_cols(
                    W_uq[:, DH * hg + NOPE:DH * hg + DH])
        wot = np.ascontiguousarray(W_o[:, RQ * g:RQ * (g + 1)].T)
        per_g.append({
            "w_uq_a": wuqa, "w_uq_b": wuqb,
            "w_ukv_n": wun, "w_ukv_v": wuv, "w_o_t": wot,
        })

    in_maps = []
    for core in range(8):
        b, g = core // 2, core % 2
        m = {"xt": np.ascontiguousarray(feats[b].T)}
        m.update(shared)
        m.update(per_g[g])
        in_maps.append(m)
    return in_maps


def kernel(**inputs):
    if "nc" not in _PROG_CACHE:
        _PROG_CACHE["nc"] = _build_program()
    nc = _PROG_CACHE["nc"]
    in_maps = _host_prep(inputs)
    res = run_bass_kernel_spmd(nc, in_maps, core_ids=list(range(8))).results
    B = 4
    out = np.empty((B, S, D), np.float32)
    ckv = np.empty((B, S, RKV + ROPE), np.float32)
    for b in range(B):
        out[b] = res[2 * b]["out_p"] + res[2 * b + 1]["out_p"]
        ckv[b] = res[2 * b]["ckv_t"].T
    return out, ckv


# revision 8
# speedup vs baseline: 199.7187x; 199.7187x over previous
"""Multi-Head Latent Attention (MLA) Trainium2 Bass kernel.

Sharding: 8 cores = (batch b in 0..4) x (head-group g in 0..2).
Each core computes one batch's full low-rank down-projections (duplicated
across the pair), its 8 heads' up-projections + attention, and a partial
W_o row-parallel output.  Host sums the two partials per batch and
transposes compressed_kv back to natural layout.

All activations live TRANSPOSED on-chip ([feature partitions, seq free]),
weights stay stationary, matmuls run in fp32r (tf32-like, full PE rate at
N=512).  RoPE's rotate_half is folded into duplicated/sign-permuted weight
columns so everything stays partition-aligned; softmax runs max-free
(scores are tiny) in the transposed layout with the denominator produced
by an extra ones-column in V.
"""

import math

import numpy as np

import concourse.bass as bass
import concourse.tile as tile
from concourse import bacc, mybir
from concourse.bass import ts
from concourse.bass_utils import run_bass_kernel_spmd

F32 = mybir.dt.float32
F32R = mybir.dt.float32r
AF = mybir.ActivationFunctionType

P = 128
S = 1024
D = 1536
RQ = 768          # q low-rank dim
RKV = 1024        # kv low-rank dim
NOPE = 48
ROPE = 48
RH = 24           # rope half
DH = 96
HL = 8            # heads per core
NB = 2            # 512-wide seq blocks
KT = 8            # 128-wide key tiles
DO = 12           # D // 128
MQ = 6            # RQ // 128
MKV = 8           # RKV // 128
LN_EPS = 1e-5
SCALE = 1.0 / math.sqrt(DH)

_PROG_CACHE = {}


def _build_program(reps=1):
    nc = bacc.Bacc("TRN2", target_bir_lowering=False, debug=False)

    def din(name, shape):
        return nc.dram_tensor(name, shape, F32, kind="ExternalInput").ap()

    xt = din("xt", [D, S])
    w_dq = din("w_dq", [D, RQ])
    w_uq_a = din("w_uq_a", [RQ, HL * 112])
    w_uq_b = din("w_uq_b", [RQ, 4 * 112])
    w_dkv_e = din("w_dkv_e", [D, 1136])
    w_ukv_n = din("w_ukv_n", [RKV, HL * 64])
    w_ukv_v = din("w_ukv_v", [RKV, HL * 96])
    w_o_t = din("w_o_t", [RQ, D])
    q_ln_w_t = din("q_ln_w_t", [P, MQ])
    q_ln_b_t = din("q_ln_b_t", [P, MQ])
    kv_ln_w_t = din("kv_ln_w_t", [P, MKV])
    kv_ln_b_t = din("kv_ln_b_t", [P, MKV])
    cos_t = din("cos_t", [P, S])
    sin_t = din("sin_t", [P, S])
    mask_t = din("mask_t", [P, 4 * 512])

    out_p = nc.dram_tensor("out_p", [S, D], F32, kind="ExternalOutput").ap()
    ckv_t = nc.dram_tensor("ckv_t", [RKV + ROPE, S], F32, kind="ExternalOutput").ap()

    with tile.TileContext(nc) as tc:
      for _rep in range(reps):
        # ---------------- constants ----------------
        constp = tc.alloc_tile_pool(name="const", bufs=1)
        cos_sb = constp.tile([P, S], F32)
        nc.sync.dma_start(cos_sb[:], cos_t[:])
        sin_sb = constp.tile([P, S], F32)
        nc.sync.dma_start(sin_sb[:], sin_t[:])
        mask_sb = constp.tile([P, 4, 512], F32)
        nc.sync.dma_start(mask_sb[:], mask_t.rearrange("p (d n) -> p d n", d=4))
        lnqw = constp.tile([P, MQ], F32)
        nc.sync.dma_start(lnqw[:], q_ln_w_t[:])
        lnqb = constp.tile([P, MQ], F32)
        nc.sync.dma_start(lnqb[:], q_ln_b_t[:])
        lnkw = constp.tile([P, MKV], F32)
        nc.sync.dma_start(lnkw[:], kv_ln_w_t[:])
        lnkb = constp.tile([P, MKV], F32)
        nc.sync.dma_start(lnkb[:], kv_ln_b_t[:])
        ones_f = constp.tile([P, 1], F32)
        nc.vector.memset(ones_f[:], 1.0)
        ones_r = constp.tile([P, 1], F32R)
        nc.vector.tensor_copy(ones_r[:], ones_f[:])
        eps_t = constp.tile([1, 1], F32)
        nc.vector.memset(eps_t[:], LN_EPS)

        # persistent activations
        ckvp = tc.alloc_tile_pool(name="ckv", bufs=1)
        ckv = ckvp.tile([P, 9, S], F32R)       # rows m<8: kv latent; m=8: rope
        krp = tc.alloc_tile_pool(name="krope", bufs=1)
        kr_a = krp.tile([112, S], F32R)        # [64:112] = raw rope-A of K
        kr_b = krp.tile([112, S], F32R)        # [64:112] = raw rope-B of K

        xtp = tc.alloc_tile_pool(name="xt", bufs=1, side="right")
        xts = xtp.tile([P, DO, S], F32R)
        for o in range(DO):
            nc.gpsimd.dma_start(xts[:, o, :], xt[P * o:P * o + P, :])

        psdp = tc.alloc_tile_pool(name="psdp", bufs=3, space="PSUM")

        # ---------------- phase A: cq = (X @ W_dq)^T ---------------------
        cqp = tc.alloc_tile_pool(name="cq", bufs=1)
        cq = cqp.tile([P, MQ, S], F32R)
        wqp = tc.alloc_tile_pool(name="wdq", bufs=1, side="right")
        wdq = wqp.tile([P, DO, RQ], F32R)
        for o in range(DO):
            nc.gpsimd.dma_start(wdq[:, o, :], w_dq[P * o:P * o + P, :])
        for m in range(MQ):
            for n in range(NB):
                ps = psdp.tile([P, 512], F32, tag="dp")
                for o in range(DO):
                    nc.tensor.matmul(
                        ps[:], wdq[:, o, ts(m, 128)], xts[:, o, ts(n, 512)],
                        start=(o == 0), stop=(o == DO - 1))
                nc.vector.tensor_copy(cq[:, m, ts(n, 512)], ps[:])
        wqp.release()

        # ---------------- phase B: ckv = (X @ W_dkv)^T -------------------
        wp = tc.alloc_tile_pool(name="wdkv", bufs=1, side="right")
        wdkv = wp.tile([P, DO, 1136], F32R)
        for o in range(DO):
            nc.gpsimd.dma_start(wdkv[:, o, :], w_dkv_e[P * o:P * o + P, :])
        for m in range(9):
            rows = 128 if m < 8 else 112
            for n in range(NB):
                ps = psdp.tile([P, 512], F32, tag="dp")
                for o in range(DO):
                    nc.tensor.matmul(
                        ps[0:rows], wdkv[:, o, 128 * m:128 * m + rows],
                        xts[:, o, ts(n, 512)], start=(o == 0), stop=(o == DO - 1))
                if m < 8:
                    nc.vector.tensor_copy(ckv[0:rows, m, ts(n, 512)], ps[0:rows])
                else:
                    nc.vector.tensor_copy(ckv[0:48, 8, ts(n, 512)], ps[0:48])
                    nc.scalar.copy(kr_a[64:112, ts(n, 512)], ps[0:48])
                    nc.scalar.copy(kr_b[64:112, ts(n, 512)], ps[64:112])
        # compressed_kv out (pre-LN, rope raw)
        for m in range(MKV):
            nc.sync.dma_start(ckv_t[P * m:P * m + P, :], ckv[:, m, :].bitcast(F32))
        nc.sync.dma_start(ckv_t[RKV:RKV + ROPE, :], ckv[0:48, 8, :].bitcast(F32))

        wp.release()
        xtp.release()
        psdp.release()

        # ---------------- layernorm (transposed, partition-reduce) ------
        lntmp = tc.alloc_tile_pool(name="lntmp", bufs=2)
        lnbc = tc.alloc_tile_pool(name="lnbc", bufs=1)
        lnvec = tc.alloc_tile_pool(name="lnvec", bufs=1)
        psst = tc.alloc_tile_pool(name="psst", bufs=1, space="PSUM")

        def ln_t(act, M, wln, bln):
            ps_sum = [psst.tile([1, 512], F32, tag=f"st{n}", name=f"st{n}") for n in range(NB)]
            ps_sq = [psst.tile([1, 512], F32, tag=f"sq{n}", name=f"sq{n}") for n in range(NB)]
            for m in range(M):
                for n in range(NB):
                    sq = lntmp.tile([P, 512], F32R, tag="sq")
                    nc.vector.tensor_mul(sq[:], act[:, m, ts(n, 512)],
                                         act[:, m, ts(n, 512)])
                    nc.tensor.matmul(ps_sum[n][:], ones_r[:], act[:, m, ts(n, 512)],
                                     start=(m == 0), stop=(m == M - 1))
                    nc.tensor.matmul(ps_sq[n][:], ones_r[:], sq[:],
                                     start=(m == 0), stop=(m == M - 1))
            mean = lnvec.tile([1, S], F32, tag="mean")
            var = lnvec.tile([1, S], F32, tag="var")
            msq = lnvec.tile([1, S], F32, tag="msq")
            for n in range(NB):
                nc.vector.tensor_scalar_mul(mean[:, ts(n, 512)], ps_sum[n][:], 1.0 / (M * P))
                nc.vector.tensor_scalar_mul(var[:, ts(n, 512)], ps_sq[n][:], 1.0 / (M * P))
            nc.vector.tensor_mul(msq[:], mean[:], mean[:])
            nc.vector.tensor_sub(var[:], var[:], msq[:])
            nc.scalar.activation(var[:], var[:], AF.Sqrt, bias=eps_t[:])
            nc.vector.reciprocal(var[:], var[:])
            mb = lnbc.tile([P, S], F32, tag="mb")
            rb = lnbc.tile([P, S], F32, tag="rb")
            nc.gpsimd.partition_broadcast(mb[:], mean[:])
            nc.gpsimd.partition_broadcast(rb[:], var[:])
            for m in range(M):
                t = lntmp.tile([P, S], F32, tag="lnt")
                nc.vector.tensor_sub(t[:], act[:, m, :], mb[:])
                nc.vector.tensor_mul(t[:], t[:], rb[:])
                nc.scalar.activation(act[:, m, :], t[:], AF.Identity,
                                     scale=wln[:, m:m + 1], bias=bln[:, m:m + 1])

        ln_t(cq, MQ, lnqw, lnqb)          # q latent LN (in place)
        ln_t(ckv, MKV, lnkw, lnkb)        # kv latent LN (in place, after ckv out)

        lnvec.release()
        lnbc.release()
        lntmp.release()
        psst.release()

        # ---------------- phase D: Q up-proj + rope ----------------------
        qkp = tc.alloc_tile_pool(name="qk", bufs=1, side="right")
        q_sb = qkp.tile([112, HL, S], F32R)
        k_sb = qkp.tile([112, HL, S], F32R)

        psq = tc.alloc_tile_pool(name="psq", bufs=3, space="PSUM")
        wup = tc.alloc_tile_pool(name="wuq", bufs=1, side="right")
        wuqa = wup.tile([P, MQ, HL * 112], F32R)
        for o in range(MQ):
            nc.gpsimd.dma_start(wuqa[:, o, :], w_uq_a[P * o:P * o + P, :])
        wuqb = wup.tile([P, MQ, 4 * 112], F32R)
        for o in range(MQ):
            nc.gpsimd.dma_start(wuqb[:, o, :], w_uq_b[P * o:P * o + P, :])

        qtmp = tc.alloc_tile_pool(name="qtmp", bufs=3, side="right")
        for h in range(HL):
            for n in range(NB):
                ps = psq.tile([P, 512], F32, tag="up")
                for o in range(MQ):
                    nc.tensor.matmul(ps[0:112], wuqa[:, o, 112 * h:112 * h + 112],
                                     cq[:, o, ts(n, 512)],
                                     start=(o == 0), stop=(o == MQ - 1))
                nc.vector.tensor_copy(q_sb[0:64, h, ts(n, 512)], ps[0:64])
                nc.vector.tensor_mul(q_sb[64:112, h, ts(n, 512)], ps[64:112],
                                     cos_sb[64:112, ts(n, 512)])
        for p4 in range(4):
            for n in range(NB):
                ps = psq.tile([P, 512], F32, tag="up")
                for o in range(MQ):
                    nc.tensor.matmul(ps[0:112], wuqb[:, o, 112 * p4:112 * p4 + 112],
                                     cq[:, o, ts(n, 512)],
                                     start=(o == 0), stop=(o == MQ - 1))
                for hh, base in ((2 * p4, 0), (2 * p4 + 1, 64)):
                    tb = qtmp.tile([112, 512], F32R, tag="tb")
                    nc.vector.tensor_mul(tb[64:112, :], ps[base:base + 48],
                                         sin_sb[64:112, ts(n, 512)])
                    nc.vector.tensor_add(q_sb[64:112, hh, ts(n, 512)],
                                         q_sb[64:112, hh, ts(n, 512)], tb[64:112, :])
        qtmp.release()
        wup.release()
        cqp.release()

        # ---------------- phase F: K rope ---------------------------------
        nc.vector.tensor_mul(kr_a[64:112, :], kr_a[64:112, :], cos_sb[64:112, :])
        nc.vector.tensor_mul(kr_b[64:112, :], kr_b[64:112, :], sin_sb[64:112, :])
        nc.vector.tensor_add(kr_a[64:112, :], kr_a[64:112, :], kr_b[64:112, :])
        for h in range(HL):
            nc.vector.tensor_copy(k_sb[64:112, h, :], kr_a[64:112, :])
        krp.release()

        # ---------------- phase G: K_nope up-proj -------------------------
        wnp = tc.alloc_tile_pool(name="wun", bufs=1, side="right")
        wun = wnp.tile([P, MKV, HL * 64], F32R)
        for o in range(MKV):
            nc.gpsimd.dma_start(wun[:, o, :], w_ukv_n[P * o:P * o + P, :])
        for mt in range(4):
            for n in range(NB):
                ps = psq.tile([P, 512], F32, tag="up")
                for o in range(MKV):
                    nc.tensor.matmul(ps[:], wun[:, o, ts(mt, 128)],
                                     ckv[:, o, ts(n, 512)],
                                     start=(o == 0), stop=(o == MKV - 1))
                nc.vector.tensor_copy(k_sb[0:64, 2 * mt, ts(n, 512)], ps[0:64])
                nc.vector.tensor_copy(k_sb[0:64, 2 * mt + 1, ts(n, 512)], ps[64:128])
        wnp.release()

        # ---------------- phase H: V up-proj (natural layout) -------------
        vp = tc.alloc_tile_pool(name="v", bufs=1, side="right")
        v_sb = vp.tile([P, KT, HL * 97], F32R)
        wvp = tc.alloc_tile_pool(name="wuv", bufs=1, side="right")
        wuv = wvp.tile([P, MKV, HL * 96], F32R)
        for o in range(MKV):
            nc.gpsimd.dma_start(wuv[:, o, :], w_ukv_v[P * o:P * o + P, :])
        for t in range(KT):
            for h in range(HL):
                nc.vector.tensor_copy(v_sb[:, t, 97 * h + 96:97 * h + 97], ones_f[:])
            for c0, cn in ((0, 512), (512, 256)):
                ps = psq.tile([P, 512], F32, tag="up")
                for o in range(MKV):
                    nc.tensor.matmul(ps[:, 0:cn], ckv[:, o, ts(t, 128)],
                                     wuv[:, o, c0:c0 + cn],
                                     start=(o == 0), stop=(o == MKV - 1))
                for h in range(HL):
                    s0, s1 = max(96 * h, c0), min(96 * h + 96, c0 + cn)
                    if s0 < s1:
                        nc.vector.tensor_copy(
                            v_sb[:, t, 97 * h + s0 - 96 * h:97 * h + s1 - 96 * h],
                            ps[:, s0 - c0:s1 - c0])
        wvp.release()
        ckvp.release()
        psq.release()

        # ---------------- phase I: attention ------------------------------
        pssc = tc.alloc_tile_pool(name="pssc", bufs=3, space="PSUM")
        pspv = tc.alloc_tile_pool(name="pspv", bufs=2, space="PSUM")
        atp = tc.alloc_tile_pool(name="attnT", bufs=1)
        attnT = atp.tile([P, MQ, S], F32R)
        ppool = tc.alloc_tile_pool(name="p", bufs=6)
        denp = tc.alloc_tile_pool(name="den", bufs=3)
        for h in range(HL):
            for j in range(NB):
                pv = pspv.tile([P, 512], F32, tag="pv")
                nkt = 4 * (j + 1)
                for t in range(nkt):
                    sp = pssc.tile([P, 512], F32, tag="s")
                    nc.tensor.matmul(sp[:], k_sb[0:112, h, ts(t, 128)],
                                     q_sb[0:112, h, ts(j, 512)],
                                     start=True, stop=True)
                    pt = ppool.tile([P, 512], F32R, tag="p")
                    nc.scalar.activation(pt[:], sp[:], AF.Exp, scale=SCALE)
                    d = t - 4 * j
                    if d >= 0:
                        nc.vector.tensor_mul(pt[:], pt[:], mask_sb[:, d, :])
                    nc.tensor.matmul(pv[0:97], v_sb[:, t, 97 * h:97 * h + 97], pt[:],
                                     start=(t == 0), stop=(t == nkt - 1))
                den = denp.tile([1, 512], F32, tag="den")
                nc.vector.tensor_copy(den[:], pv[96:97, :])
                nc.vector.reciprocal(den[:], den[:])
                rb = denp.tile([P, 512], F32, tag="rb")
                nc.gpsimd.partition_broadcast(rb[:], den[:])
                f0 = 96 * h
                for z in range(3):          # 32-row pieces keep all partition
                    fs = f0 + 32 * z        # bases 32-aligned and counts <= 32
                    nc.vector.tensor_mul(
                        attnT[fs % 128:fs % 128 + 32, fs // 128, ts(j, 512)],
                        pv[32 * z:32 * z + 32, :], rb[fs % 128:fs % 128 + 32, :])

        denp.release()
        ppool.release()
        pspv.release()
        vp.release()
        qkp.release()

        # ---------------- phase J: out = attnT.T @ W_o^T partial ---------
        wop = tc.alloc_tile_pool(name="wo", bufs=1, side="right")
        wot = wop.tile([P, MQ, D], F32R)
        for o in range(MQ):
            nc.gpsimd.dma_start(wot[:, o, :], w_o_t[P * o:P * o + P, :])
        stout = tc.alloc_tile_pool(name="stout", bufs=3)
        for m in range(8):
            for n in range(3):
                ps = pssc.tile([P, 512], F32, tag="s")
                for o in range(MQ):
                    nc.tensor.matmul(ps[:], attnT[:, o, ts(m, 128)],
                                     wot[:, o, ts(n, 512)],
                                     start=(o == 0), stop=(o == MQ - 1))
                st = stout.tile([P, 512], F32, tag="st")
                nc.vector.tensor_copy(st[:], ps[:])
                nc.sync.dma_start(out_p[ts(m, 128), ts(n, 512)], st[:])
        stout.release()
        wop.release()
        atp.release()
        pssc.release()
        constp.release()

    nc.compile()
    return nc


def _rot_cols(a):
    return np.concatenate([-a[:, RH:2 * RH], a[:, 0:RH]], axis=1)


def _host_prep(inputs):
    feats = np.ascontiguousarray(np.asarray(inputs["features"], np.float32))
    W_dq = np.asarray(inputs["W_dq"], np.float32)
    W_uq = np.asarray(inputs["W_uq"], np.float32)
    q_ln_w = np.asarray(inputs["q_ln_w"], np.float32)
    q_ln_b = np.asarray(inputs["q_ln_b"], np.float32)
    W_dkv = np.asarray(inputs["W_dkv"], np.float32)
    W_ukv = np.asarray(inputs["W_ukv"], np.float32)
    kv_ln_w = np.asarray(inputs["kv_ln_w"], np.float32)
    kv_ln_b = np.asarray(inputs["kv_ln_b"], np.float32)
    W_o = np.asarray(inputs["W_o"], np.float32)
    cos_tab = np.asarray(inputs["cos_cached"], np.float32)[0, 0, :S, 0:RH]
    sin_tab = np.asarray(inputs["sin_cached"], np.float32)[0, 0, :S, 0:RH]

    cos_t = np.zeros((P, S), np.float32)
    sin_t = np.zeros((P, S), np.float32)
    jj = (np.arange(64, 112) - 64) % RH
    cos_t[64:112] = cos_tab[:, jj].T
    sin_t[64:112] = sin_tab[:, jj].T

    f = np.arange(512)
    p = np.arange(P)
    mask_t = np.concatenate(
        [(f[None, :] >= (p[:, None] + 128 * d)).astype(np.float32) for d in range(4)],
        axis=1)

    def part_major(v, m):
        return np.ascontiguousarray(v.reshape(m, P).T)

    wdkv_e = np.zeros((D, 1136), np.float32)
    wdkv_e[:, :RKV] = W_dkv[:, :RKV]
    wdkv_e[:, RKV:RKV + ROPE] = W_dkv[:, RKV:RKV + ROPE]
    wdkv_e[:, 1088:1136] = _rot_cols(W_dkv[:, RKV:RKV + ROPE])

    shared = {
        "w_dq": np.ascontiguousarray(W_dq),
        "w_dkv_e": wdkv_e,
        "q_ln_w_t": part_major(q_ln_w, MQ),
        "q_ln_b_t": part_major(q_ln_b, MQ),
        "kv_ln_w_t": part_major(kv_ln_w, MKV),
        "kv_ln_b_t": part_major(kv_ln_b, MKV),
        "cos_t": cos_t,
        "sin_t": sin_t,
        "mask_t": np.ascontiguousarray(mask_t),
    }

    per_g = []
    for g in range(2):
        h0 = HL * g
        wuqa = np.zeros((RQ, HL * 112), np.float32)
        wuqb = np.zeros((RQ, 4 * 112), np.float32)
        wun = np.zeros((RKV, HL * 64), np.float32)
        wuv = np.zeros((RKV, HL * 96), np.float32)
        for h in range(HL):
            hg = h0 + h
            wuqa[:, 112 * h:112 * h + NOPE] = W_uq[:, DH * hg:DH * hg + NOPE]
            wuqa[:, 112 * h + 64:112 * h + 112] = W_uq[:, DH * hg + NOPE:DH * hg + DH]
            wun[:, 64 * h:64 * h + NOPE] = W_ukv[:, 144 * hg:144 * hg + NOPE]
            wuv[:, 96 * h:96 * h + 96] = W_ukv[:, 144 * hg + NOPE:144 * hg + 144]
        for p4 in range(4):
            for k, h in enumerate((2 * p4, 2 * p4 + 1)):
                hg = h0 + h
                wuqb[:, 112 * p4 + 64 * k:112 * p4 + 64 * k + 48] = _rot_cols(
                    W_uq[:, DH * hg + NOPE:DH * hg + DH])
        wot = np.ascontiguousarray(W_o[:, RQ * g:RQ * (g + 1)].T)
        per_g.append({
            "w_uq_a": wuqa, "w_uq_b": wuqb,
            "w_ukv_n": wun, "w_ukv_v": wuv, "w_o_t": wot,
        })

    in_maps = []
    for core in range(8):
        b, g = core // 2, core % 2
        m = {"xt": np.ascontiguousarray(feats[b].T)}
        m.update(shared)
        m.update(per_g[g])
        in_maps.append(m)
    return in_maps


def kernel(**inputs):
    if "nc" not in _PROG_CACHE:
        _PROG_CACHE["nc"] = _build_program()
    nc = _PROG_CACHE["nc"]
    in_maps = _host_prep(inputs)
    res = run_bass_kernel_spmd(nc, in_maps, core_ids=list(range(8))).results
    B = 4
    out = np.empty((B, S, D), np.float32)
    ckv = np.empty((B, S, RKV + ROPE), np.float32)
    for b in range(B):
        out[b] = res[2 * b]["out_p"] + res[2 * b + 1]["out_p"]
        ckv[b] = res[2 * b]["ckv_t"].T
    return out, ckv


# revision 17
# speedup vs baseline: 245.5560x; 1.2295x over previous
"""Multi-Head Latent Attention (MLA) Trainium2 Bass kernel.

Sharding: 8 cores = (batch b in 0..4) x (head-group g in 0..2).
Each core computes one batch's full low-rank down-projections (duplicated
across the pair), its 8 heads' up-projections + attention, and a partial
W_o row-parallel output.  Host sums the two partials per batch and
transposes compressed_kv back to natural layout.

All activations live TRANSPOSED on-chip ([feature partitions, seq free]),
weights stay stationary, matmuls run in fp32r (tf32-like, full PE rate at
N=512).  RoPE's rotate_half is folded into duplicated/sign-permuted weight
columns so everything stays partition-aligned; softmax runs max-free
(scores are tiny) in the transposed layout with the denominator produced
by an extra ones-column in V.
"""

import math

import numpy as np

import concourse.bass as bass
import concourse.tile as tile
from concourse import bacc, mybir
from concourse.bass import ts
from concourse.bass_utils import run_bass_kernel_spmd

F32 = mybir.dt.float32
F32R = mybir.dt.float32r
AF = mybir.ActivationFunctionType

P = 128
S = 1024
D = 1536
RQ = 768          # q low-rank dim
RKV = 1024        # kv low-rank dim
NOPE = 48
ROPE = 48
RH = 24           # rope half
DH = 96
HL = 8            # heads per core
NB = 2            # 512-wide seq blocks
KT = 8            # 128-wide key tiles
DO = 12           # D // 128
MQ = 6            # RQ // 128
MKV = 8           # RKV // 128
LN_EPS = 1e-5
SCALE = 1.0 / math.sqrt(DH)

_PROG_CACHE = {}


def _build_program(reps=1):
    nc = bacc.Bacc("TRN2", target_bir_lowering=False, debug=False)

    def din(name, shape, dt=F32):
        return nc.dram_tensor(name, shape, dt, kind="ExternalInput").ap()

    xt = din("xt", [D, S], F32R)
    w_dq = din("w_dq", [D, RQ], F32R)
    w_uq_a = din("w_uq_a", [RQ, HL * 112], F32R)
    w_uq_b = din("w_uq_b", [RQ, 4 * 112], F32R)
    w_dkv_e = din("w_dkv_e", [D, 1136], F32R)
    w_ukv_n = din("w_ukv_n", [RKV, HL * 64], F32R)
    w_ukv_v = din("w_ukv_v", [RKV, HL * 96], F32R)
    w_o_t = din("w_o_t", [RQ, D], F32R)
    q_ln_w_t = din("q_ln_w_t", [P, MQ])
    q_ln_b_t = din("q_ln_b_t", [P, MQ])
    kv_ln_w_t = din("kv_ln_w_t", [P, MKV])
    kv_ln_b_t = din("kv_ln_b_t", [P, MKV])
    cos_t = din("cos_t", [P, S])
    sin_t = din("sin_t", [P, S])
    mask_t = din("mask_t", [P, P])

    out_p = nc.dram_tensor("out_p", [S, D], F32, kind="ExternalOutput").ap()
    ckv_t = nc.dram_tensor("ckv_t", [RKV + ROPE, S], F32, kind="ExternalOutput").ap()

    with tile.TileContext(nc) as tc:
      for _rep in range(reps):
        # ---------------- constants ----------------
        constp = tc.alloc_tile_pool(name="const", bufs=1)
        lnqw = constp.tile([P, MQ], F32)
        nc.sync.dma_start(lnqw[:], q_ln_w_t[:])
        lnqb = constp.tile([P, MQ], F32)
        nc.sync.dma_start(lnqb[:], q_ln_b_t[:])
        lnkw = constp.tile([P, MKV], F32)
        nc.sync.dma_start(lnkw[:], kv_ln_w_t[:])
        lnkb = constp.tile([P, MKV], F32)
        nc.sync.dma_start(lnkb[:], kv_ln_b_t[:])
        ones_f = constp.tile([P, 1], F32)
        nc.vector.memset(ones_f[:], 1.0)
        ones_r = constp.tile([P, 1], F32R)
        nc.vector.tensor_copy(ones_r[:], ones_f[:])
        eps_t = constp.tile([1, 1], F32)
        nc.vector.memset(eps_t[:], LN_EPS)

        # persistent activations
        ckvp = tc.alloc_tile_pool(name="ckv", bufs=1)
        ckv = ckvp.tile([P, 9, S], F32R)       # rows m<8: kv latent; m=8: rope
        krp = tc.alloc_tile_pool(name="krope", bufs=1)
        kr_a = krp.tile([112, S], F32R)        # [64:112] = raw rope-A of K
        kr_b = krp.tile([112, S], F32R)        # [64:112] = raw rope-B of K

        xtp = tc.alloc_tile_pool(name="xt", bufs=1, side="right")
        xts = xtp.tile([P, DO, S], F32R)

        psdp = tc.alloc_tile_pool(name="psdp", bufs=3, space="PSUM")

        # ---------------- phase A: cq = (X @ W_dq)^T ---------------------
        cqp = tc.alloc_tile_pool(name="cq", bufs=1)
        cq = cqp.tile([P, MQ, S], F32R)
        wqp = tc.alloc_tile_pool(name="wdq", bufs=1, side="right")
        wdq = wqp.tile([P, DO, RQ], F32R)
        for o in range(DO):
            nc.sync.dma_start(wdq[:, o, :], w_dq[P * o:P * o + P, :])
            nc.sync.dma_start(xts[:, o, :], xt[P * o:P * o + P, :])
        for m in range(MQ):
            for n in range(NB):
                ps = psdp.tile([P, 512], F32, tag="dp")
                for o in range(DO):
                    nc.tensor.matmul(
                        ps[:], wdq[:, o, ts(m, 128)], xts[:, o, ts(n, 512)],
                        start=(o == 0), stop=(o == DO - 1))
                nc.scalar.copy(cq[:, m, ts(n, 512)], ps[:])
        wqp.release()

        # ---------------- layernorm helpers ------------------------------
        lntmp = tc.alloc_tile_pool(name="lntmp", bufs=2)
        lnbc = tc.alloc_tile_pool(name="lnbc", bufs=1)
        lnvec = tc.alloc_tile_pool(name="lnvec", bufs=2)
        psst = tc.alloc_tile_pool(name="psst", bufs=1, space="PSUM")

        def ln_stats(act, M):
            ps_sum = [psst.tile([1, 512], F32, tag=f"st{n}", name=f"st{n}") for n in range(NB)]
            ps_sq = [psst.tile([1, 512], F32, tag=f"sq{n}", name=f"sq{n}") for n in range(NB)]
            for m in range(M):
                for n in range(NB):
                    sq = lntmp.tile([P, 512], F32R, tag="sq")
                    nc.scalar.square(sq[:], act[:, m, ts(n, 512)])
                    nc.tensor.matmul(ps_sum[n][:], ones_r[:], act[:, m, ts(n, 512)],
                                     start=(m == 0), stop=(m == M - 1))
                    nc.tensor.matmul(ps_sq[n][:], ones_r[:], sq[:],
                                     start=(m == 0), stop=(m == M - 1))
            mean = lnvec.tile([1, S], F32, tag="mean")
            var = lnvec.tile([1, S], F32, tag="var")
            for n in range(NB):
                msq = lnvec.tile([1, 512], F32, tag="msq")
                nc.vector.tensor_scalar_mul(mean[:, ts(n, 512)], ps_sum[n][:], 1.0 / (M * P))
                nc.vector.tensor_scalar_mul(var[:, ts(n, 512)], ps_sq[n][:], 1.0 / (M * P))
                nc.vector.tensor_mul(msq[:], mean[:, ts(n, 512)], mean[:, ts(n, 512)])
                nc.vector.tensor_sub(var[:, ts(n, 512)], var[:, ts(n, 512)], msq[:])
            nc.scalar.activation(var[:], var[:], AF.Sqrt, bias=eps_t[:])
            nc.vector.reciprocal(var[:], var[:])
            return mean, var

        def ln_apply(act, M, wln, bln, mean, rstd):
            mb = lnbc.tile([P, S], F32, tag="mb")
            rb = lnbc.tile([P, S], F32, tag="rb")
            nc.gpsimd.partition_broadcast(mb[:], mean[:])
            nc.gpsimd.partition_broadcast(rb[:], rstd[:])
            for m in range(M):
                for n in range(NB):
                    t = lntmp.tile([P, 512], F32, tag="lnt")
                    nc.vector.tensor_sub(t[:], act[:, m, ts(n, 512)], mb[:, ts(n, 512)])
                    nc.vector.tensor_mul(t[:], t[:], rb[:, ts(n, 512)])
                    nc.scalar.activation(act[:, m, ts(n, 512)], t[:], AF.Identity,
                                         scale=wln[:, m:m + 1], bias=bln[:, m:m + 1])

        q_mean, q_rstd = ln_stats(cq, MQ)

        # ---------------- phase B: ckv = (X @ W_dkv)^T -------------------
        wp = tc.alloc_tile_pool(name="wdkv", bufs=2, side="right")
        for c0, ms in ((0, (0, 1, 2)), (384, (3, 4, 5)), (768, (6, 7, 8))):
            cw = 384 if c0 < 768 else 368
            wdkv = wp.tile([P, DO, 384], F32R, tag="wdkv", name="wdkv")
            for o in range(DO):
                nc.sync.dma_start(wdkv[:, o, 0:cw],
                                  w_dkv_e[P * o:P * o + P, c0:c0 + cw])
            for m in ms:
                rows = 128 if m < 8 else 112
                for n in range(NB):
                    ps = psdp.tile([P, 512], F32, tag="dp")
                    for o in range(DO):
                        nc.tensor.matmul(
                            ps[0:rows],
                            wdkv[:, o, 128 * m - c0:128 * m - c0 + rows],
                            xts[:, o, ts(n, 512)], start=(o == 0), stop=(o == DO - 1))
                    if m < 8:
                        nc.scalar.copy(ckv[0:rows, m, ts(n, 512)], ps[0:rows])
                    else:
                        nc.scalar.copy(ckv[0:48, 8, ts(n, 512)], ps[0:48])
                        nc.scalar.copy(kr_a[64:112, ts(n, 512)], ps[0:48])
                        nc.scalar.copy(kr_b[64:112, ts(n, 512)], ps[64:112])
        wp.release()
        xtp.release()

        # (ln pools were allocated before phase B; applies emitted here)
        ln_apply(cq, MQ, lnqw, lnqb, q_mean, q_rstd)
        kv_mean, kv_rstd = ln_stats(ckv, MKV)
        # compressed_kv out (pre-LN values) — must be emitted before the
        # in-place kv LN apply; gpsimd queue keeps sync free for weights
        for m in range(MKV):
            nc.gpsimd.dma_start(ckv_t[P * m:P * m + P, :], ckv[:, m, :].bitcast(F32))
        nc.gpsimd.dma_start(ckv_t[RKV:RKV + ROPE, :], ckv[0:48, 8, :].bitcast(F32))
        ln_apply(ckv, MKV, lnkw, lnkb, kv_mean, kv_rstd)

        lnvec.release()
        lnbc.release()
        lntmp.release()
        psst.release()
        psdp.release()

        # ---------------- phase D: Q up-proj + rope ----------------------
        csp = tc.alloc_tile_pool(name="cossin", bufs=1)
        cos_sb = csp.tile([P, S], F32)
        nc.sync.dma_start(cos_sb[:], cos_t[:])
        sin_sb = csp.tile([P, S], F32)
        nc.sync.dma_start(sin_sb[:], sin_t[:])
        qkp = tc.alloc_tile_pool(name="qk", bufs=1, side="right")
        q_sb = qkp.tile([112, HL, S], F32R)
        k_sb = qkp.tile([112, HL, S], F32R)

        psq = tc.alloc_tile_pool(name="psq", bufs=3, space="PSUM")
        wnp = tc.alloc_tile_pool(name="wun", bufs=1, side="right")
        wun = wnp.tile([P, MKV, HL * 64], F32R)
        wup = tc.alloc_tile_pool(name="wuq", bufs=1, side="right")
        wuqa = wup.tile([P, MQ, HL * 112], F32R)
        for o in range(MQ):
            nc.sync.dma_start(wuqa[:, o, :], w_uq_a[P * o:P * o + P, :])
        wuqb = wup.tile([P, MQ, 4 * 112], F32R)
        for o in range(MQ):
            nc.sync.dma_start(wuqb[:, o, :], w_uq_b[P * o:P * o + P, :])
        for o in range(MKV):
            nc.sync.dma_start(wun[:, o, :], w_ukv_n[P * o:P * o + P, :])

        qtmp = tc.alloc_tile_pool(name="qtmp", bufs=2, side="right")
        for h in range(HL):
            for n in range(NB):
                ps = psq.tile([P, 512], F32, tag="up")
                for o in range(MQ):
                    nc.tensor.matmul(ps[0:112], wuqa[:, o, 112 * h:112 * h + 112],
                                     cq[:, o, ts(n, 512)],
                                     start=(o == 0), stop=(o == MQ - 1))
                nc.vector.tensor_copy(q_sb[0:64, h, ts(n, 512)], ps[0:64])
                nc.vector.tensor_mul(q_sb[64:112, h, ts(n, 512)], ps[64:112],
                                     cos_sb[64:112, ts(n, 512)])
        for p4 in range(4):
            for n in range(NB):
                ps = psq.tile([P, 512], F32, tag="up")
                for o in range(MQ):
                    nc.tensor.matmul(ps[0:112], wuqb[:, o, 112 * p4:112 * p4 + 112],
                                     cq[:, o, ts(n, 512)],
                                     start=(o == 0), stop=(o == MQ - 1))
                for hh, base in ((2 * p4, 0), (2 * p4 + 1, 64)):
                    tb = qtmp.tile([112, 512], F32R, tag="tb")
                    nc.vector.tensor_mul(tb[64:112, :], ps[base:base + 48],
                                         sin_sb[64:112, ts(n, 512)])
                    nc.vector.tensor_add(q_sb[64:112, hh, ts(n, 512)],
                                         q_sb[64:112, hh, ts(n, 512)], tb[64:112, :])
        qtmp.release()
        wup.release()

        # ---------------- phase F: K rope ---------------------------------
        nc.vector.tensor_mul(kr_a[64:112, :], kr_a[64:112, :], cos_sb[64:112, :])
        nc.vector.tensor_mul(kr_b[64:112, :], kr_b[64:112, :], sin_sb[64:112, :])
        nc.vector.tensor_add(kr_a[64:112, :], kr_a[64:112, :], kr_b[64:112, :])
        for h in range(HL):
            nc.vector.tensor_copy(k_sb[64:112, h, :], kr_a[64:112, :])
        csp.release()
        cqp.release()
        krp.release()

        # ---------------- phase G: K_nope up-proj -------------------------
        for mt in range(4):
            for n in range(NB):
                ps = psq.tile([P, 512], F32, tag="up")
                for o in range(MKV):
                    nc.tensor.matmul(ps[:], wun[:, o, ts(mt, 128)],
                                     ckv[:, o, ts(n, 512)],
                                     start=(o == 0), stop=(o == MKV - 1))
                nc.vector.tensor_copy(k_sb[0:64, 2 * mt, ts(n, 512)], ps[0:64])
                nc.vector.tensor_copy(k_sb[0:64, 2 * mt + 1, ts(n, 512)], ps[64:128])
        wnp.release()

        # ---------------- phase H: V up-proj (natural layout) -------------
        vp = tc.alloc_tile_pool(name="v", bufs=1, side="right")
        v_sb = vp.tile([P, KT, HL * 97], F32R)
        wvp = tc.alloc_tile_pool(name="wuv", bufs=1, side="right")
        wuv = wvp.tile([P, MKV, HL * 96], F32R)
        for o in range(MKV):
            nc.sync.dma_start(wuv[:, o, :], w_ukv_v[P * o:P * o + P, :])
        for t in range(KT):
            for h in range(HL):
                nc.vector.tensor_copy(v_sb[:, t, 97 * h + 96:97 * h + 97], ones_f[:])
            vv = v_sb[:, t, :].rearrange("p (h c) -> p h c", c=97)
            for c0, cn in ((0, 480), (480, 288)):
                ps = psq.tile([P, 512], F32, tag="up")
                for o in range(MKV):
                    nc.tensor.matmul(ps[:, 0:cn], ckv[:, o, ts(t, 128)],
                                     wuv[:, o, c0:c0 + cn],
                                     start=(o == 0), stop=(o == MKV - 1))
                h0 = c0 // 96
                nc.vector.tensor_copy(
                    vv[:, h0:h0 + cn // 96, 0:96],
                    ps[:, 0:cn].rearrange("p (h c) -> p h c", c=96))
        wvp.release()
        ckvp.release()
        psq.release()

        # ---------------- phase I: attention ------------------------------
        pssc = tc.alloc_tile_pool(name="pssc", bufs=4, space="PSUM")
        pspv = tc.alloc_tile_pool(name="pspv", bufs=3, space="PSUM")
        atp = tc.alloc_tile_pool(name="attnT", bufs=1)
        attnT = atp.tile([P, MQ, S], F32R)
        mkp = tc.alloc_tile_pool(name="mask", bufs=1)
        mask_sb = mkp.tile([P, P], F32)
        nc.sync.dma_start(mask_sb[:], mask_t[:])
        ppool = tc.alloc_tile_pool(name="p", bufs=8)
        denp = tc.alloc_tile_pool(name="den", bufs=3)
        for h in range(HL):
            for j in range(NB):
                pv = pspv.tile([P, 512], F32, tag="pv")
                nkt = 4 * (j + 1)
                for w0 in range(0, nkt, 4):
                    pts = []
                    for t in range(w0, w0 + 4):
                        d = t - 4 * j
                        c0 = max(0, 128 * d)      # first possibly-valid column
                        sp = pssc.tile([P, 512], F32, tag="s", name=f"s{t}")
                        nc.tensor.matmul(sp[:, c0:512], k_sb[0:112, h, ts(t, 128)],
                                         q_sb[0:112, h, 512 * j + c0:512 * (j + 1)],
                                         start=True, stop=True)
                        pt = ppool.tile([P, 512], F32R, tag="p", name=f"p{t}")
                        nc.scalar.activation(pt[:, c0:512], sp[:, c0:512],
                                             AF.Exp, scale=SCALE)
                        if d >= 0:
                            nc.vector.tensor_mul(pt[:, c0:c0 + 128],
                                                 pt[:, c0:c0 + 128], mask_sb[:])
                        pts.append((pt, c0))
                    for t in range(w0, w0 + 4):
                        pt, c0 = pts[t - w0]
                        nc.tensor.matmul(pv[0:97, c0:512],
                                         v_sb[:, t, 97 * h:97 * h + 97],
                                         pt[:, c0:512],
                                         start=(t == 0), stop=(t == nkt - 1))
                den = denp.tile([1, 512], F32, tag="den")
                nc.vector.reciprocal(den[:], pv[96:97, :])
                rb = denp.tile([P, 512], F32, tag="rb")
                nc.gpsimd.partition_broadcast(rb[:], den[:])
                f0 = 96 * h
                for z in range(3):          # 32-row pieces keep all partition
                    fs = f0 + 32 * z        # bases 32-aligned and counts <= 32
                    nc.vector.tensor_mul(
                        attnT[fs % 128:fs % 128 + 32, fs // 128, ts(j, 512)],
                        pv[32 * z:32 * z + 32, :], rb[fs % 128:fs % 128 + 32, :])

        denp.release()
        ppool.release()
        mkp.release()
        pspv.release()
        vp.release()
        qkp.release()

        # ---------------- phase J: out = attnT.T @ W_o^T partial ---------
        wop = tc.alloc_tile_pool(name="wo", bufs=1, side="right")
        wot = wop.tile([P, MQ, D], F32R)
        for o in range(MQ):
            nc.sync.dma_start(wot[:, o, :], w_o_t[P * o:P * o + P, :])
        stout = tc.alloc_tile_pool(name="stout", bufs=3)
        for m in range(8):
            for n in range(3):
                ps = pssc.tile([P, 512], F32, tag="s")
                for o in range(MQ):
                    nc.tensor.matmul(ps[:], attnT[:, o, ts(m, 128)],
                                     wot[:, o, ts(n, 512)],
                                     start=(o == 0), stop=(o == MQ - 1))
                st = stout.tile([P, 512], F32, tag="st")
                nc.scalar.copy(st[:], ps[:])
                nc.sync.dma_start(out_p[ts(m, 128), ts(n, 512)], st[:])
        stout.release()
        wop.release()
        atp.release()
        pssc.release()
        constp.release()

    nc.compile()
    return nc


def _rot_cols(a):
    return np.concatenate([-a[:, RH:2 * RH], a[:, 0:RH]], axis=1)


def _host_prep(inputs):
    feats = np.ascontiguousarray(np.asarray(inputs["features"], np.float32))
    W_dq = np.asarray(inputs["W_dq"], np.float32)
    W_uq = np.asarray(inputs["W_uq"], np.float32)
    q_ln_w = np.asarray(inputs["q_ln_w"], np.float32)
    q_ln_b = np.asarray(inputs["q_ln_b"], np.float32)
    W_dkv = np.asarray(inputs["W_dkv"], np.float32)
    W_ukv = np.asarray(inputs["W_ukv"], np.float32)
    kv_ln_w = np.asarray(inputs["kv_ln_w"], np.float32)
    kv_ln_b = np.asarray(inputs["kv_ln_b"], np.float32)
    W_o = np.asarray(inputs["W_o"], np.float32)
    cos_tab = np.asarray(inputs["cos_cached"], np.float32)[0, 0, :S, 0:RH]
    sin_tab = np.asarray(inputs["sin_cached"], np.float32)[0, 0, :S, 0:RH]

    cos_t = np.zeros((P, S), np.float32)
    sin_t = np.zeros((P, S), np.float32)
    jj = (np.arange(64, 112) - 64) % RH
    cos_t[64:112] = cos_tab[:, jj].T
    sin_t[64:112] = sin_tab[:, jj].T

    f = np.arange(P)
    p = np.arange(P)
    mask_t = (f[None, :] >= p[:, None]).astype(np.float32)

    def part_major(v, m):
        return np.ascontiguousarray(v.reshape(m, P).T)

    wdkv_e = np.zeros((D, 1136), np.float32)
    wdkv_e[:, :RKV] = W_dkv[:, :RKV]
    wdkv_e[:, RKV:RKV + ROPE] = W_dkv[:, RKV:RKV + ROPE]
    wdkv_e[:, 1088:1136] = _rot_cols(W_dkv[:, RKV:RKV + ROPE])

    shared = {
        "w_dq": np.ascontiguousarray(W_dq),
        "w_dkv_e": wdkv_e,
        "q_ln_w_t": part_major(q_ln_w, MQ),
        "q_ln_b_t": part_major(q_ln_b, MQ),
        "kv_ln_w_t": part_major(kv_ln_w, MKV),
        "kv_ln_b_t": part_major(kv_ln_b, MKV),
        "cos_t": cos_t,
        "sin_t": sin_t,
        "mask_t": np.ascontiguousarray(mask_t),
    }

    per_g = []
    for g in range(2):
        h0 = HL * g
        wuqa = np.zeros((RQ, HL * 112), np.float32)
        wuqb = np.zeros((RQ, 4 * 112), np.float32)
        wun = np.zeros((RKV, HL * 64), np.float32)
        wuv = np.zeros((RKV, HL * 96), np.float32)
        for h in range(HL):
            hg = h0 + h
            wuqa[:, 112 * h:112 * h + NOPE] = W_uq[:, DH * hg:DH * hg + NOPE]
            wuqa[:, 112 * h + 64:112 * h + 112] = W_uq[:, DH * hg + NOPE:DH * hg + DH]
            wun[:, 64 * h:64 * h + NOPE] = W_ukv[:, 144 * hg:144 * hg + NOPE]
            wuv[:, 96 * h:96 * h + 96] = W_ukv[:, 144 * hg + NOPE:144 * hg + 144]
        for p4 in range(4):
            for k, h in enumerate((2 * p4, 2 * p4 + 1)):
                hg = h0 + h
                wuqb[:, 112 * p4 + 64 * k:112 * p4 + 64 * k + 48] = _rot_cols(
                    W_uq[:, DH * hg + NOPE:DH * hg + DH])
        wot = np.ascontiguousarray(W_o[:, RQ * g:RQ * (g + 1)].T)
        per_g.append({
            "w_uq_a": wuqa, "w_uq_b": wuqb,
            "w_ukv_n": wun, "w_ukv_v": wuv, "w_o_t": wot,
        })

    in_maps = []
    for core in range(8):
        b, g = core // 2, core % 2
        m = {"xt": np.ascontiguousarray(feats[b].T)}
        m.update(shared)
        m.update(per_g[g])
        in_maps.append(m)
    return in_maps


def kernel(**inputs):
    if "nc" not in _PROG_CACHE:
        _PROG_CACHE["nc"] = _build_program()
    nc = _PROG_CACHE["nc"]
    in_maps = _host_prep(inputs)
    res = run_bass_kernel_spmd(nc, in_maps, core_ids=list(range(8))).results
    B = 4
    out = np.empty((B, S, D), np.float32)
    ckv = np.empty((B, S, RKV + ROPE), np.float32)
    for b in range(B):
        out[b] = res[2 * b]["out_p"] + res[2 * b + 1]["out_p"]
        ckv[b] = res[2 * b]["ckv_t"].T
    return out, ckv
